# revision 19
# baseline (speedup 1.0000x reference)
"""LocallyConnected2dV2 Trainium2 kernel (bf16, raw pre-context input stream).

Math: out[b, a, bp] = sum_{k,l} xpad[b, 5a+k, 5bp+l] * kw[a, bp, k, l] + bias[a, bp]

Strategy (8 cores, data-parallel over batch, 128 images/core):
  - Host: transpose each core's x shard to [col j', row r, batch b]; cast
    bf16; compact W into per-image-row banded blocks Wh[r, j', 50] (bf16).
  - Input stream: ALL 8 combined x+w chunk DMAs plus the aux (bias/ones)
    DMA are raw-bass instructions issued BEFORE the TileContext: they
    bypass the ~1.2us context-entry prologue and drain strictly FIFO on
    the two HWDGE rings (sync: even chunks; scalar: aux + odd chunks) —
    no Tile DMA scheduling, no issue-lane stalls.
  - PE gating: NX dispatches in order, so a tiny constant dummy matmul
    carrying a semaphore wait (attached post-scheduling — the Tile sim
    cannot model the external DMAs) gates every later Ldweights/Matmult.
    One gate is emitted right before the first consumer of each chunk.
  - Compute: per output-row group g (5 x 125 psum cols), accumulate
    ~30 row matmuls (bf16, fp32 psum); bias enters last via a K=1
    ones-vector matmul; DVE casts psum->bf16; per-group output DMA.
"""

import numpy as np
import ml_dtypes

BF16 = ml_dtypes.bfloat16

B = 1024
R = 128           # image rows = cols
NCORES = 8
BS = B // NCORES  # 128 batch per core
NK = 625
WP = 132
NG = 5            # output-row groups (5 a's each)
GW = 125          # psum cols per group
CHUNK = 16        # image rows per DMA chunk
NCH = R // CHUNK


def _a0_of_row(r):
    return min(max((r - 3) // 5, 0), 23)


def _group_rows(g):
    return range(max(0, 25 * g - 2), min(R - 1, 25 * g + 27) + 1)


def _row_parts(r, g):
    """Matmul pieces row r contributes to group g."""
    a0 = _a0_of_row(r)
    lo = 5 * g
    if a0 >= lo and a0 + 1 < lo + 5:
        return [((a0 - lo) * 25, 50, 0)]
    parts = []
    for ai, a in ((0, a0), (1, a0 + 1)):
        if lo <= a < lo + 5:
            parts.append(((a - lo) * 25, 25, ai * 25))
    return parts


def prep_weights(W, bias):
    """W [17424, 625], bias [25,25] -> wt [128, 128*50], bs [1, 625]."""
    W = np.asarray(W, np.float32)
    i = np.arange(NK)
    si = (i // 25) * 5
    sj = (i % 25) * 5
    rows = ((si[:, None, None] + np.arange(10)[None, :, None]) * WP
            + sj[:, None, None] + np.arange(10)[None, None, :])
    kw = W[rows.reshape(NK, 100), i[:, None]].reshape(25, 25, 10, 10)

    r = np.arange(R)[:, None, None]
    jp = np.arange(R)[None, :, None]
    c = np.arange(50)[None, None, :]
    ai = c // 25
    bp = c % 25
    a = np.clip((r - 3) // 5, 0, 23) + ai
    k = r + 2 - 5 * a
    l = jp + 2 - 5 * bp
    valid = (k >= 0) & (k < 10) & (l >= 0) & (l < 10)
    Wh = np.where(valid, kw[a, bp, np.clip(k, 0, 9), np.clip(l, 0, 9)], 0.0)
    Wh = Wh.astype(np.float32)                       # [r, j', 50]
    wt = np.ascontiguousarray(Wh.transpose(1, 0, 2)).reshape(R, R * 50)
    bs = np.ascontiguousarray(np.asarray(bias, np.float32).reshape(1, NK))
    return wt, bs


CW = CHUNK * BS + CHUNK * 50   # combined x+w free cols per chunk


def _build_nc():
    import concourse.bass as bass
    import concourse.mybir as mybir
    import concourse.tile as tile
    from concourse import bacc

    bf16 = mybir.dt.bfloat16
    nc = bacc.Bacc("TRN2", target_bir_lowering=False, debug=False)
    xw = nc.dram_tensor("xw", [R, NCH * CW], bf16, kind="ExternalInput").ap()
    aux = nc.dram_tensor("aux", [1, NK + BS], bf16, kind="ExternalInput").ap()
    out = nc.dram_tensor("out", [NG * BS, GW], bf16, kind="ExternalOutput").ap()

    # raw pre-context input stream
    es = [nc.alloc_sbuf_tensor(f"e{i}", [R, CW], bf16) for i in range(NCH)]
    aux_sb = nc.alloc_sbuf_tensor("auxsb", [1, NK + BS], bf16)
    sems = [nc.alloc_semaphore(f"esem{i}") for i in range(NCH)]
    saux = nc.alloc_semaphore("sauxsem")
    # ring order — sync: ch0, aux, ch2, ch4, ch6; scalar: ch1, ch3, ch5, ch7.
    # aux rides the sync ring behind ch0 (delivered ~12us, first needed ~18us)
    # so the scalar ring starts streaming ch1 with no issue delay ahead of it.
    def chunk_dma(i):
        eng = nc.sync if i % 2 == 0 else nc.scalar
        eng.dma_start(es[i].ap(), xw[:, i * CW:(i + 1) * CW]).then_inc(
            sems[i], 16)
    chunk_dma(0)
    chunk_dma(1)
    nc.sync.dma_start(aux_sb.ap(), aux[:]).then_inc(saux, 16)
    for i in range(2, NCH):
        chunk_dma(i)

    bias_t = aux_sb.ap()[:, 0:NK]
    ones_t = aux_sb.ap()[:, NK:NK + BS]

    gates = []   # (dummy matmul inst, semaphore) — waits attached post-sched

    with tile.TileContext(nc) as tc:
        with (
            tc.tile_pool(name="ps", bufs=5, space=bass.MemorySpace.PSUM) as ps_pool,
            tc.tile_pool(name="dps", bufs=1, space=bass.MemorySpace.PSUM) as dps_pool,
            tc.tile_pool(name="ob", bufs=1) as ob_pool,
        ):
            one_bf = nc.const_aps.aps[(mybir.dt.bfloat16, 1.0)]
            dps = dps_pool.tile([1, 1], mybir.dt.float32, tag="dummy")

            def gate(sem):
                inst = nc.tensor.matmul(
                    dps[0:1, 0:1], one_bf[0:1, 0:1], one_bf[0:1, 0:1],
                    start=True, stop=True, skip_group_check=True)
                gates.append((inst, sem))

            seen = set()
            out_sb = ob_pool.tile([BS, NK], bf16, tag="osb")
            for g in range(NG):
                ps = ps_pool.tile([BS, GW], mybir.dt.float32, tag="ps")
                started = False
                for r in _group_rows(g):
                    ic = r // CHUNK
                    if ic not in seen:
                        gate(sems[ic])
                        seen.add(ic)
                    ct = es[ic].ap()
                    lhsT = ct[:, (r % CHUNK) * BS:(r % CHUNK + 1) * BS]
                    wb = CHUNK * BS + (r % CHUNK) * 50
                    for (pc, n, wc) in _row_parts(r, g):
                        nc.tensor.matmul(ps[:, pc:pc + n], lhsT,
                                         ct[:, wb + wc:wb + wc + n],
                                         start=not started, stop=False)
                        started = True
                # bias enters last (stop matmul of the group)
                if "aux" not in seen:
                    gate(saux)
                    seen.add("aux")
                nc.tensor.matmul(ps[:, 0:GW], ones_t,
                                 bias_t[:, g * GW:(g + 1) * GW],
                                 start=False, stop=True)
                nc.vector.tensor_copy(
                    out_sb[:, g * GW:(g + 1) * GW], ps[:])
                nc.scalar.dma_start(out[g * BS:(g + 1) * BS, :],
                                    out_sb[:, g * GW:(g + 1) * GW])

    # Attach the stream waits post-scheduling: the Tile simulator does not
    # model the pre-context DMAs and would report a false deadlock.
    for inst, sem in gates:
        inst._wait_ge(sem, 16)
    nc.compile()
    return nc


_NC_CACHE = []


def _get_nc():
    if not _NC_CACHE:
        _NC_CACHE.append(_build_nc())
    return _NC_CACHE[0]


def make_in_maps(x, W, bias):
    x = np.asarray(x, np.float32)
    wt, bsv = prep_weights(W, bias)
    wt16 = wt.astype(BF16)
    auxv = np.concatenate(
        [bsv.astype(BF16), np.ones((1, BS), BF16)], axis=1)
    in_maps = []
    for c in range(NCORES):
        xc = x[c * BS:(c + 1) * BS]                      # [b, r, j']
        xtv = np.ascontiguousarray(
            xc.transpose(2, 1, 0)).astype(BF16).reshape(R, R * BS)
        parts = []
        for ic in range(NCH):
            parts.append(xtv[:, ic * CHUNK * BS:(ic + 1) * CHUNK * BS])
            parts.append(wt16[:, ic * CHUNK * 50:(ic + 1) * CHUNK * 50])
        xwv = np.ascontiguousarray(np.concatenate(parts, axis=1))
        in_maps.append({"xw": xwv, "aux": auxv})
    return in_maps


def run(x, W, bias, trace=False, **kw):
    from concourse import bass_utils
    nc = _get_nc()
    res = bass_utils.run_bass_kernel_spmd(
        nc, make_in_maps(x, W, bias), list(range(NCORES)), trace=trace, **kw)
    outs = []
    for c in range(NCORES):
        o = np.asarray(res.results[c]["out"])            # [NG*BS, GW] bf16
        o = o.reshape(NG, BS, GW).transpose(1, 0, 2)     # [BS, NG, GW]
        outs.append(o.reshape(BS, 25, 25).astype(np.float32))
    return np.concatenate(outs, axis=0), res


def kernel(**inputs):
    out, _ = run(inputs["x"], inputs["W"], inputs["bias"])
    return out
